# revision 5
# baseline (speedup 1.0000x reference)
"""ACSL loss kernel for 8 TRN2 NeuronCores (Bass/Tile, data-parallel over rows).

Reference math (row i, col c, n_c=1204, bg col=1203, THR=logit(0.7)):
  loss_el = softplus(x) - x * onehot(label)
  weight:  fg rows: max([x>=THR], onehot) ; bg rows: [sel_rand < colthr[c]]
  out = sum(weight * loss_el) / n_i

Decomposition (t = max(x-THR, 0), computed over ALL rows incl. bg):
  [x>=THR]*sp(x) = t + THR*[t>0] + f(t),  f(t) = ln(1+e^(-t-THR))
  f(t) ~= A_F*e^(-B_F*t) + C_F*t + D_F  (weighted LSQ fit, rel err ~1e-7 on
  the randn input distribution; distribution-free bound ~2e-3).
  Sum over counted elements handled via 3 sums over ALL elements:
    R = sum t, C = sum [t>0], E = sum e^(-B_F*t)
  main = R + THR*C + A_F*E + C_F*R + D_F*N - (N - C)*(A_F + D_F)
  The ~7 bg rows/core stream through the main pass too; their exact R/C/E
  contribution is recomputed on the small host-gathered xbg tile and
  subtracted, then their true bg loss is added (exact Exp+Ln softplus).
  Label-col forcing for fg rows with g=x[i,label] < THR adds sp(g); the
  -x*target term subtracts sum(g) over all rows.

Engine mapping per core (64 blocks of [128,1204]):
  gpsimd SWDGE DMA: 2 blocks per transfer, f32 -> bf16 cast in-flight
  (halves SBUF-port write traffic; DVE gets 2x-rate bf16 inputs).
  DVE: per 2 blocks, scalar_tensor_tensor (x-THR) max 0 -> t (accum: R col)
  and (t is_gt 0) add 0 (accum: C col).  ACT: Exp(-B_F*t) with accum (E col)
  over 4-block groups, 2-block taper at both ends.  No PE matmuls: all
  reductions ride the DVE/ACT accumulators into a [128,87] f32 tile that is
  DMA'd out whole; host does the final cross-column/partition reduction.
"""

import math

import numpy as np

N_I = 65536
N_C = 1204
NUM_CLASSES = 1203
N_CORES = 8
RPC = N_I // N_CORES          # rows per core
NBLK = RPC // 128             # 64 blocks of 128 rows
NB2 = NBLK // 2               # 32 two-block units
THR = math.log(0.7 / 0.3)     # logit(0.7)
C_SP = math.log(1.0 / 0.3)    # softplus(THR)
BG_PAD = 32                   # bg-row slots per core (mean ~7, 32 is ~10 sigma)

# f(t) = ln(1+e^(-t-THR)) ~= A_F*exp(-B_F*t) + C_F*t + D_F  (t >= 0)
A_F = 0.39617708
B_F = 0.79508084
C_F = 0.0066877854
D_F = -0.038736005

# accumulator columns in the [128, NCOL] acc tile
COL_R = 0                     # 32 cols: sum t, one per 2-block unit
COL_C = 32                    # 32 cols: count t>0
COL_E = 64                    # 17 cols: sum e^(-B_F t), one per ACT group
N_EGRP = 17
COL_RBG = 81
COL_CBG = 82
COL_EBG = 83
COL_BGT = 84                  # exact bg loss term
COL_CORR = 85                 # fg label-col forcing
COL_GSUM = 86                 # sum of label-col logits
NCOL = 87

USE_CAST = True               # SWDGE f32->bf16 cast on the main x DMAs

_CACHE = {}


def _build_nc():
    import concourse.bacc as bacc
    import concourse.tile as tile
    from concourse import mybir

    f32 = mybir.dt.float32
    bf16 = mybir.dt.bfloat16

    nc = bacc.Bacc(
        "TRN2",
        target_bir_lowering=False,
        debug=False,
        enable_asserts=True,
        num_devices=N_CORES,
    )

    x = nc.dram_tensor("x", [RPC, N_C], f32, kind="ExternalInput").ap()
    xbg = nc.dram_tensor("xbg", [BG_PAD, N_C], f32, kind="ExternalInput").ap()
    bg_sel = nc.dram_tensor("bg_sel", [BG_PAD, 1], f32, kind="ExternalInput").ap()
    colthr = nc.dram_tensor("colthr", [BG_PAD, N_C], f32, kind="ExternalInput").ap()
    gv = nc.dram_tensor("gv", [128, NBLK], f32, kind="ExternalInput").ap()
    fgm = nc.dram_tensor("fgm", [128, NBLK], f32, kind="ExternalInput").ap()
    out = nc.dram_tensor("out", [128, NCOL], f32, kind="ExternalOutput").ap()

    W2 = 2 * N_C  # 2408

    with tile.TileContext(nc) as tc:
        with (
            tc.tile_pool(name="const", bufs=1) as const,
            tc.tile_pool(name="xp", bufs=10 if USE_CAST else 6) as xp,
            tc.tile_pool(name="tp", bufs=3) as tp,
            tc.tile_pool(name="t2p", bufs=2) as t2p,
            tc.tile_pool(name="scr", bufs=2) as scr,
            tc.tile_pool(name="sidep", bufs=1) as sidep,
        ):
            AF = mybir.ActivationFunctionType
            OP = mybir.AluOpType

            xdt = bf16 if USE_CAST else f32
            big = nc.gpsimd if USE_CAST else nc.sync
            small = nc.sync if USE_CAST else nc.gpsimd

            xr = x.rearrange("(n p) c -> n p c", p=128)

            def x_dma(b2, x2_t):
                big.dma_start(
                    out=x2_t[:, :, :],
                    in_=xr[2 * b2 : 2 * b2 + 2, :, :].rearrange("n p c -> p n c"),
                )

            # first two x transfers issued before anything else
            x2_head = []
            for b2 in range(2):
                x2_t = xp.tile([128, 2, N_C], xdt, tag="x2")
                x_dma(b2, x2_t)
                x2_head.append(x2_t)

            zeros2 = const.tile([128, W2], bf16)
            nc.vector.memset(zeros2[:, :], 0.0)
            acc = const.tile([128, NCOL], f32)
            nc.vector.memset(acc[:, :], 0.0)

            # small inputs on the other DMA path
            bg_sel_sb = const.tile([BG_PAD, 1], f32)
            small.dma_start(out=bg_sel_sb[:, :], in_=bg_sel)
            colthr_sb = const.tile([BG_PAD, N_C], f32)
            small.dma_start(out=colthr_sb[:, :], in_=colthr)
            xbg_t = sidep.tile([BG_PAD, N_C], f32)
            small.dma_start(out=xbg_t[:, :], in_=xbg)
            g_t = const.tile([128, NBLK], f32)
            small.dma_start(out=g_t[:, :], in_=gv)
            fgm_sb = const.tile([128, NBLK], f32)
            small.dma_start(out=fgm_sb[:, :], in_=fgm)

            # --- side pass (runs in the DMA ramp shadow) ---
            # exact bg loss: sum_bg sum_c [sel < colthr_c] * sp(x)
            ebg_t = sidep.tile([BG_PAD, N_C], f32)
            nc.scalar.activation(ebg_t[:, :], xbg_t[:, :], AF.Exp)
            spbg_t = sidep.tile([BG_PAD, N_C], bf16)
            nc.scalar.activation(spbg_t[:, :], ebg_t[:, :], AF.Ln, bias=1.0)
            bgp_t = sidep.tile([BG_PAD, N_C], bf16)
            nc.vector.scalar_tensor_tensor(
                out=bgp_t[:, :], in0=colthr_sb[:, :], scalar=bg_sel_sb[:, :1],
                in1=spbg_t[:, :], op0=OP.is_gt, op1=OP.mult,
                accum_out=acc[:BG_PAD, COL_BGT : COL_BGT + 1],
            )
            # bg rows' main-pass contribution (to subtract): R_bg, C_bg, E_bg
            tbg_t = sidep.tile([BG_PAD, N_C], bf16)
            nc.vector.scalar_tensor_tensor(
                out=tbg_t[:, :], in0=xbg_t[:, :], scalar=float(THR),
                in1=zeros2[:BG_PAD, :N_C], op0=OP.subtract, op1=OP.max,
                accum_out=acc[:BG_PAD, COL_RBG : COL_RBG + 1],
            )
            cbg_t = sidep.tile([BG_PAD, N_C], bf16)
            nc.vector.scalar_tensor_tensor(
                out=cbg_t[:, :], in0=tbg_t[:, :], scalar=0.0,
                in1=zeros2[:BG_PAD, :N_C], op0=OP.is_gt, op1=OP.add,
                accum_out=acc[:BG_PAD, COL_CBG : COL_CBG + 1],
            )
            exbg_t = sidep.tile([BG_PAD, N_C], bf16)
            nc.scalar.activation(
                exbg_t[:, :], tbg_t[:, :], AF.Exp, scale=float(-B_F),
                accum_out=acc[:BG_PAD, COL_EBG : COL_EBG + 1],
            )
            # fg label-col forcing: sum sp(g)*[sp(g) < C_SP]*fgm ; and sum g
            eg_t = const.tile([128, NBLK], f32)
            nc.scalar.activation(eg_t[:, :], g_t[:, :], AF.Exp)
            spg_t = const.tile([128, NBLK], f32)
            nc.scalar.activation(spg_t[:, :], eg_t[:, :], AF.Ln, bias=1.0)
            mf_t = const.tile([128, NBLK], f32)
            nc.vector.scalar_tensor_tensor(
                out=mf_t[:, :], in0=spg_t[:, :], scalar=float(C_SP),
                in1=fgm_sb[:, :], op0=OP.is_lt, op1=OP.mult,
            )
            cpr_t = const.tile([128, NBLK], f32)
            nc.vector.scalar_tensor_tensor(
                out=cpr_t[:, :], in0=mf_t[:, :], scalar=1.0,
                in1=spg_t[:, :], op0=OP.mult, op1=OP.mult,
                accum_out=acc[:, COL_CORR : COL_CORR + 1],
            )
            gsc_t = const.tile([128, NBLK], f32)
            nc.vector.tensor_scalar(
                out=gsc_t[:, :], in0=g_t[:, :],
                scalar1=1.0, scalar2=None, op0=OP.mult, op1=OP.add,
                accum_out=acc[:, COL_GSUM : COL_GSUM + 1],
            )

            # --- main loop ---
            # ACT groups: 2-block taper at both ends, 4-block middle
            plan = [[0]] + [[1 + 2 * i, 2 + 2 * i] for i in range(15)] + [[31]]

            def op_ab(b2, x2_t, t_out):
                # t = max(x - THR, 0), accum R; cnt = sum [t>0], accum C
                nc.vector.scalar_tensor_tensor(
                    out=t_out, in0=x2_t[:, :, :].rearrange("p a c -> p (a c)"),
                    scalar=float(THR), in1=zeros2[:, :],
                    op0=OP.subtract, op1=OP.max,
                    accum_out=acc[:, COL_R + b2 : COL_R + b2 + 1],
                )
                scrB = scr.tile([128, W2], bf16, tag="scrB")
                nc.vector.scalar_tensor_tensor(
                    out=scrB[:, :], in0=t_out, scalar=0.0,
                    in1=zeros2[:, :], op0=OP.is_gt, op1=OP.add,
                    accum_out=acc[:, COL_C + b2 : COL_C + b2 + 1],
                )

            for gi, b2s in enumerate(plan):
                if len(b2s) == 1:
                    b2 = b2s[0]
                    x2_t = x2_head[b2] if b2 < 2 else xp.tile(
                        [128, 2, N_C], xdt, tag="x2"
                    )
                    if b2 >= 2:
                        x_dma(b2, x2_t)
                    t2_t = t2p.tile([128, W2], bf16, tag="t2")
                    op_ab(b2, x2_t, t2_t[:, :])
                    ex2_t = scr.tile([128, W2], bf16, tag="ex2")
                    nc.scalar.activation(
                        ex2_t[:, :], t2_t[:, :], AF.Exp, scale=float(-B_F),
                        accum_out=acc[:, COL_E + gi : COL_E + gi + 1],
                    )
                else:
                    t4_t = tp.tile([128, 2, W2], bf16, tag="t4")
                    for j, b2 in enumerate(b2s):
                        x2_t = x2_head[b2] if b2 < 2 else xp.tile(
                            [128, 2, N_C], xdt, tag="x2"
                        )
                        if b2 >= 2:
                            x_dma(b2, x2_t)
                        op_ab(b2, x2_t, t4_t[:, j, :])
                    ex4_t = scr.tile([128, 2, W2], bf16, tag="ex4")
                    nc.scalar.activation(
                        ex4_t[:, :, :].rearrange("p a c -> p (a c)"),
                        t4_t[:, :, :].rearrange("p a c -> p (a c)"),
                        AF.Exp, scale=float(-B_F),
                        accum_out=acc[:, COL_E + gi : COL_E + gi + 1],
                    )

            # ship the whole accumulator tile; host reduces
            nc.sync.dma_start(out=out, in_=acc[:, :])

    nc.compile()
    return nc


def _get_nc():
    if "nc" not in _CACHE:
        _CACHE["nc"] = _build_nc()
    return _CACHE["nc"]


def _prep_inputs(cls_logits, labels, sel_rand, cat_freq):
    """Host-side shard + small index-tensor prep (O(n_i + n_c) work)."""
    cls_logits = np.ascontiguousarray(cls_logits, dtype=np.float32)
    labels = np.asarray(labels, dtype=np.int32)
    sel_rand = np.asarray(sel_rand, dtype=np.int32)
    cat_freq = np.asarray(cat_freq, dtype=np.int32)

    bg = labels == NUM_CLASSES  # [N_I]

    colthr = np.empty(N_C, dtype=np.float32)
    colthr[:NUM_CLASSES] = np.choose(cat_freq, [10.0, 100.0, 1000.0])
    colthr[NUM_CLASSES] = 1000.0
    colthr32 = np.ascontiguousarray(
        np.broadcast_to(colthr.reshape(1, N_C), (BG_PAD, N_C))
    )

    in_maps = []
    for core in range(N_CORES):
        sl = slice(core * RPC, (core + 1) * RPC)
        x_sh = cls_logits[sl]
        lab_sh = labels[sl]
        bg_sh = bg[sl]
        sel_sh = sel_rand[sl]

        # [128, NBLK] layouts: tile[p, b] corresponds to shard row b*128 + p
        g = x_sh[np.arange(RPC), lab_sh]
        gv = np.ascontiguousarray(g.reshape(NBLK, 128).T)
        fgm = np.ascontiguousarray((~bg_sh).astype(np.float32).reshape(NBLK, 128).T)

        bgrows = np.nonzero(bg_sh)[0]
        assert len(bgrows) <= BG_PAD
        xbg = np.zeros((BG_PAD, N_C), dtype=np.float32)
        bg_sel = np.full((BG_PAD, 1), 2000.0, dtype=np.float32)
        xbg[: len(bgrows)] = x_sh[bgrows]
        bg_sel[: len(bgrows), 0] = sel_sh[bgrows]

        in_maps.append(
            {
                "x": x_sh,
                "xbg": xbg,
                "bg_sel": bg_sel,
                "colthr": colthr32,
                "gv": gv,
                "fgm": fgm,
                "_n_bg": len(bgrows),  # host-side only, stripped before run
            }
        )
    return in_maps


def _device_maps(in_maps):
    return [{k: v for k, v in m.items() if not k.startswith("_")} for m in in_maps]


def _combine(results, in_maps):
    total = 0.0
    for r, m in zip(results, in_maps):
        o = np.asarray(r["out"], dtype=np.float64)  # [128, NCOL]
        cols = o.sum(axis=0)
        R = cols[COL_R : COL_R + 32].sum()
        C = cols[COL_C : COL_C + 32].sum()
        E = cols[COL_E : COL_E + N_EGRP].sum()
        n_bg = m["_n_bg"]
        # strip bg rows' main-pass contribution (xbg padding rows are zeros:
        # they add 0 to R/C and exactly 1.0 per element to E_bg)
        Rf = R - cols[COL_RBG]
        Cf = C - cols[COL_CBG]
        Ef = E - (cols[COL_EBG] - (BG_PAD - n_bg) * N_C)
        Nf = (RPC - n_bg) * N_C
        term2 = A_F * Ef + C_F * Rf + D_F * Nf - (Nf - Cf) * (A_F + D_F)
        total += (
            Rf + THR * Cf + term2 + cols[COL_BGT] + cols[COL_CORR] - cols[COL_GSUM]
        )
    return np.asarray(total / N_I, dtype=np.float32)


def kernel(cls_logits, labels, sel_rand, cat_freq):
    from concourse.bass_utils import run_bass_kernel_spmd

    nc = _get_nc()
    in_maps = _prep_inputs(cls_logits, labels, sel_rand, cat_freq)
    res = run_bass_kernel_spmd(nc, _device_maps(in_maps), core_ids=list(range(N_CORES)))
    return _combine(res.results, in_maps)


# revision 10
# speedup vs baseline: 1.6182x; 1.6182x over previous
"""ACSL loss kernel for 8 TRN2 NeuronCores (Bass/Tile, data-parallel over rows).

Reference math (row i, col c, n_c=1204, bg col=1203, THR=logit(0.7)):
  loss_el = softplus(x) - x * onehot(label)
  weight:  fg rows: max([x>=THR], onehot) ; bg rows: [sel_rand < colthr[c]]
  out = sum(weight * loss_el) / n_i

Decomposition: with t = x - THR over counted elements (x > THR),
  [x>=THR]*sp(x) = t + THR + f(t),  f(t) = ln(1+e^(-t-THR))
  f(t) ~= A_F*e^(-B_F*t) + C_F*t + D_F  (weighted LSQ fit; end-to-end rel
  err ~1e-7 on the randn input distribution, distribution-free ~2e-3).
  total = R*(1+C_F) + C*(THR+D_F) + A_F*e^(B_F*THR)*E  summed over fg rows,
  where R = sum t, C = count, E = sum e^(-B_F*x) over counted elements.

Device computes, over ALL rows (bg rows subtracted via an exact side pass
on the ~7 host-gathered bg rows/core):
  u = max(x_bf16, THR)         (DVE, single-ALU op, bf16 2x rate)
  mask = [u > THR_B]           (DVE, bf16 2x; THR_B = bf16(THR))
  S = sum u, C = sum mask      (PE ones-matmuls into PSUM; DVE accum_out
                                forces 1x rate, so sums ride the PE)
  E' = sum e^(-B_F*u)          (ACT Exp accumulator, 4-block groups)
Host recovers R/C/E from S/C/E' using THR_B and two on-device calibration
columns (the ACT-exp value of the clamped constant in bf16 and f32 paths),
then applies the closed form.  x arrives via SWDGE DMA with f32->bf16 cast
in-flight (probed: bit-exact RNE, ~same stream rate as HWDGE f32), packed
2 rows per partition for contiguous 9.6KB descriptors.
"""

import math

import numpy as np

N_I = 65536
N_C = 1204
NUM_CLASSES = 1203
N_CORES = 8
RPC = N_I // N_CORES          # rows per core
NBLK = RPC // 128             # 64 blocks of 128 rows
NB2 = NBLK // 2               # 32 two-block units (256 rows each)
THR = math.log(0.7 / 0.3)     # logit(0.7)
THR_B = 0.84765625            # bf16(THR), round-to-nearest-even
C_SP = math.log(1.0 / 0.3)    # softplus(THR)
BG_PAD = 32                   # bg-row slots per core (mean ~7, 32 is ~10 sigma)
NBG = BG_PAD * N_C
N_TOT = RPC * N_C

# f(t) = ln(1+e^(-t-THR)) ~= A_F*exp(-B_F*t) + C_F*t + D_F  (t >= 0)
A_F = 0.39617708
B_F = 0.79508084
C_F = 0.0066877854
D_F = -0.038736005

# accumulator columns in the [128, NCOL] acc tile
COL_E = 0                     # 32 cols: sum e^(-B_F u), one per 2-block unit
COL_CALQ = 32                 # ACT-exp of THR_B (bf16 path), x32 per partition
COL_CALQS = 33                # ACT-exp of THR (f32 side path), x32
COL_SBG = 34
COL_CBG = 35
COL_EBG = 36
COL_BGT = 37                  # exact bg loss term
COL_CORR = 38                 # fg label-col forcing
COL_GSUM = 39                 # sum of label-col logits
NCOL = 40
NCAL = 32                     # free-dim width of the calibration tiles

_CACHE = {}


def _build_nc():
    import concourse.bacc as bacc
    import concourse.tile as tile
    from concourse import mybir

    f32 = mybir.dt.float32
    bf16 = mybir.dt.bfloat16

    nc = bacc.Bacc(
        "TRN2",
        target_bir_lowering=False,
        debug=False,
        enable_asserts=True,
        num_devices=N_CORES,
    )

    x = nc.dram_tensor("x", [RPC, N_C], f32, kind="ExternalInput").ap()
    xbg = nc.dram_tensor("xbg", [BG_PAD, N_C], f32, kind="ExternalInput").ap()
    bg_sel = nc.dram_tensor("bg_sel", [BG_PAD, 1], f32, kind="ExternalInput").ap()
    colthr = nc.dram_tensor("colthr", [BG_PAD, N_C], f32, kind="ExternalInput").ap()
    gv = nc.dram_tensor("gv", [128, NBLK], f32, kind="ExternalInput").ap()
    fgm = nc.dram_tensor("fgm", [128, NBLK], f32, kind="ExternalInput").ap()
    out = nc.dram_tensor("out", [128, NCOL], f32, kind="ExternalOutput").ap()
    out_sc = nc.dram_tensor("out_sc", [2, N_C], f32, kind="ExternalOutput").ap()

    W2 = 2 * N_C  # 2408
    SL = [(0, 512), (512, 1024), (1024, N_C)]

    with tile.TileContext(nc) as tc:
        with (
            tc.tile_pool(name="const", bufs=1) as const,
            tc.tile_pool(name="xp", bufs=10) as xp,
            tc.tile_pool(name="up", bufs=4) as up,
            tc.tile_pool(name="scr", bufs=2) as scr,
            tc.tile_pool(name="sidep", bufs=1) as sidep,
            tc.tile_pool(name="psum", bufs=1, space="PSUM") as psp,
        ):
            AF = mybir.ActivationFunctionType
            OP = mybir.AluOpType

            def x_dma(b2, x2_t):
                nc.gpsimd.dma_start(
                    out=x2_t[:, :, :],
                    in_=x[256 * b2 : 256 * (b2 + 1), :].rearrange(
                        "(p j) c -> p j c", j=2
                    ),
                )

            # first x transfers issued before anything else
            x2_head = []
            for b2 in range(3):
                x2_t = xp.tile([128, 2, N_C], bf16, tag="x2")
                x_dma(b2, x2_t)
                x2_head.append(x2_t)

            ones_bf = const.tile([128, 1], bf16)
            nc.vector.memset(ones_bf[:, :], 1.0)
            acc = const.tile([128, NCOL], f32)
            nc.vector.memset(acc[:, :], 0.0)

            psum_s = psp.tile([1, N_C], f32)
            psum_c = psp.tile([1, N_C], f32)

            # small inputs on the HWDGE path (gpsimd queue carries the x casts)
            bg_sel_sb = const.tile([BG_PAD, 1], f32)
            nc.sync.dma_start(out=bg_sel_sb[:, :], in_=bg_sel)
            colthr_sb = const.tile([BG_PAD, N_C], f32)
            nc.sync.dma_start(out=colthr_sb[:, :], in_=colthr)
            xbg_t = sidep.tile([BG_PAD, N_C], f32)
            nc.sync.dma_start(out=xbg_t[:, :], in_=xbg)
            g_t = const.tile([128, NBLK], f32)
            nc.sync.dma_start(out=g_t[:, :], in_=gv)
            fgm_sb = const.tile([128, NBLK], f32)
            nc.sync.dma_start(out=fgm_sb[:, :], in_=fgm)

            # --- calibration columns (device value of exp at the clamps) ---
            calq_t = const.tile([128, NCAL], bf16)
            nc.vector.memset(calq_t[:, :], float(THR_B))
            calq_o = const.tile([128, NCAL], bf16)
            nc.scalar.activation(
                calq_o[:, :], calq_t[:, :], AF.Exp, scale=float(-B_F),
                accum_out=acc[:, COL_CALQ : COL_CALQ + 1],
            )
            calqs_t = const.tile([128, NCAL], f32)
            nc.vector.memset(calqs_t[:, :], float(THR))
            calqs_o = const.tile([128, NCAL], f32)
            nc.scalar.activation(
                calqs_o[:, :], calqs_t[:, :], AF.Exp, scale=float(-B_F),
                accum_out=acc[:, COL_CALQS : COL_CALQS + 1],
            )

            # --- side pass (runs in the DMA ramp shadow) ---
            # exact bg loss: sum_bg sum_c [sel < colthr_c] * sp(x)
            ebg_t = sidep.tile([BG_PAD, N_C], f32)
            nc.scalar.activation(ebg_t[:, :], xbg_t[:, :], AF.Exp)
            spbg_t = sidep.tile([BG_PAD, N_C], bf16)
            nc.scalar.activation(spbg_t[:, :], ebg_t[:, :], AF.Ln, bias=1.0)
            bgp_t = sidep.tile([BG_PAD, N_C], bf16)
            nc.vector.scalar_tensor_tensor(
                out=bgp_t[:, :], in0=colthr_sb[:, :], scalar=bg_sel_sb[:, :1],
                in1=spbg_t[:, :], op0=OP.is_gt, op1=OP.mult,
                accum_out=acc[:BG_PAD, COL_BGT : COL_BGT + 1],
            )
            # bg rows' main-pass contribution (to subtract): S_bg, C_bg, E_bg
            # (f32 path; xbg padding rows are zeros -> u=THR, count 0)
            ubg_t = sidep.tile([BG_PAD, N_C], f32)
            nc.vector.tensor_scalar(
                out=ubg_t[:, :], in0=xbg_t[:, :], scalar1=float(THR),
                scalar2=None, op0=OP.max, op1=OP.add,
                accum_out=acc[:BG_PAD, COL_SBG : COL_SBG + 1],
            )
            cbg_t = sidep.tile([BG_PAD, N_C], bf16)
            nc.vector.tensor_scalar(
                out=cbg_t[:, :], in0=ubg_t[:, :], scalar1=float(THR_B),
                scalar2=None, op0=OP.is_gt, op1=OP.add,
                accum_out=acc[:BG_PAD, COL_CBG : COL_CBG + 1],
            )
            exbg_t = sidep.tile([BG_PAD, N_C], f32)
            nc.scalar.activation(
                exbg_t[:, :], ubg_t[:, :], AF.Exp, scale=float(-B_F),
                accum_out=acc[:BG_PAD, COL_EBG : COL_EBG + 1],
            )
            # fg label-col forcing: sum sp(g)*[sp(g) < C_SP]*fgm ; and sum g
            eg_t = const.tile([128, NBLK], f32)
            nc.scalar.activation(eg_t[:, :], g_t[:, :], AF.Exp)
            spg_t = const.tile([128, NBLK], f32)
            nc.scalar.activation(spg_t[:, :], eg_t[:, :], AF.Ln, bias=1.0)
            mf_t = const.tile([128, NBLK], f32)
            nc.vector.scalar_tensor_tensor(
                out=mf_t[:, :], in0=spg_t[:, :], scalar=float(C_SP),
                in1=fgm_sb[:, :], op0=OP.is_lt, op1=OP.mult,
            )
            cpr_t = const.tile([128, NBLK], f32)
            nc.vector.scalar_tensor_tensor(
                out=cpr_t[:, :], in0=mf_t[:, :], scalar=1.0,
                in1=spg_t[:, :], op0=OP.mult, op1=OP.mult,
                accum_out=acc[:, COL_CORR : COL_CORR + 1],
            )
            gsc_t = const.tile([128, NBLK], f32)
            nc.vector.tensor_scalar(
                out=gsc_t[:, :], in0=g_t[:, :],
                scalar1=1.0, scalar2=None, op0=OP.mult, op1=OP.add,
                accum_out=acc[:, COL_GSUM : COL_GSUM + 1],
            )

            # --- main loop ---
            for b2 in range(NB2):
                if b2 < 3:
                    x2_t = x2_head[b2]
                else:
                    x2_t = xp.tile([128, 2, N_C], bf16, tag="x2")
                    x_dma(b2, x2_t)
                x2f = x2_t[:, :, :].rearrange("p j c -> p (j c)")
                u2_t = up.tile([128, W2], bf16, tag="u2")
                nc.vector.tensor_scalar(
                    out=u2_t[:, :], in0=x2f, scalar1=float(THR),
                    scalar2=None, op0=OP.max,
                )
                mk_t = scr.tile([128, W2], bf16, tag="mk")
                nc.vector.tensor_scalar(
                    out=mk_t[:, :], in0=u2_t[:, :], scalar1=float(THR_B),
                    scalar2=None, op0=OP.is_gt,
                )
                ex_t = scr.tile([128, W2], bf16, tag="ex")
                nc.scalar.activation(
                    ex_t[:, :], u2_t[:, :], AF.Exp, scale=float(-B_F),
                    accum_out=acc[:, COL_E + b2 : COL_E + b2 + 1],
                )
                first = b2 == 0
                last = b2 == NB2 - 1
                for j in range(2):
                    for s0, s1 in SL:
                        nc.tensor.matmul(
                            out=psum_s[0:1, s0:s1], lhsT=ones_bf[:, :],
                            rhs=u2_t[:, j * N_C + s0 : j * N_C + s1],
                            start=(first and j == 0), stop=(last and j == 1),
                        )
                for j in range(2):
                    for s0, s1 in SL:
                        nc.tensor.matmul(
                            out=psum_c[0:1, s0:s1], lhsT=ones_bf[:, :],
                            rhs=mk_t[:, j * N_C + s0 : j * N_C + s1],
                            start=(first and j == 0), stop=(last and j == 1),
                        )

            # --- final: ship accumulators; host reduces ---
            s_sb = const.tile([1, N_C], f32)
            nc.vector.tensor_copy(out=s_sb[:, :], in_=psum_s[:, :])
            c_sb = const.tile([1, N_C], f32)
            nc.vector.tensor_copy(out=c_sb[:, :], in_=psum_c[:, :])
            nc.sync.dma_start(out=out, in_=acc[:, :])
            nc.sync.dma_start(out=out_sc[0:1, :], in_=s_sb[:, :])
            nc.sync.dma_start(out=out_sc[1:2, :], in_=c_sb[:, :])

    nc.compile()
    return nc


def _get_nc():
    if "nc" not in _CACHE:
        _CACHE["nc"] = _build_nc()
    return _CACHE["nc"]


def _prep_inputs(cls_logits, labels, sel_rand, cat_freq):
    """Host-side shard + small index-tensor prep (O(n_i + n_c) work)."""
    cls_logits = np.ascontiguousarray(cls_logits, dtype=np.float32)
    labels = np.asarray(labels, dtype=np.int32)
    sel_rand = np.asarray(sel_rand, dtype=np.int32)
    cat_freq = np.asarray(cat_freq, dtype=np.int32)

    bg = labels == NUM_CLASSES  # [N_I]

    colthr = np.empty(N_C, dtype=np.float32)
    colthr[:NUM_CLASSES] = np.choose(cat_freq, [10.0, 100.0, 1000.0])
    colthr[NUM_CLASSES] = 1000.0
    colthr32 = np.ascontiguousarray(
        np.broadcast_to(colthr.reshape(1, N_C), (BG_PAD, N_C))
    )

    in_maps = []
    for core in range(N_CORES):
        sl = slice(core * RPC, (core + 1) * RPC)
        x_sh = cls_logits[sl]
        lab_sh = labels[sl]
        bg_sh = bg[sl]
        sel_sh = sel_rand[sl]

        # [128, NBLK] layouts: tile[p, b] corresponds to shard row b*128 + p
        g = x_sh[np.arange(RPC), lab_sh]
        gv = np.ascontiguousarray(g.reshape(NBLK, 128).T)
        fgm = np.ascontiguousarray((~bg_sh).astype(np.float32).reshape(NBLK, 128).T)

        bgrows = np.nonzero(bg_sh)[0]
        assert len(bgrows) <= BG_PAD
        xbg = np.zeros((BG_PAD, N_C), dtype=np.float32)
        bg_sel = np.full((BG_PAD, 1), 2000.0, dtype=np.float32)
        xbg[: len(bgrows)] = x_sh[bgrows]
        bg_sel[: len(bgrows), 0] = sel_sh[bgrows]

        in_maps.append(
            {
                "x": x_sh,
                "xbg": xbg,
                "bg_sel": bg_sel,
                "colthr": colthr32,
                "gv": gv,
                "fgm": fgm,
            }
        )
    return in_maps


def _combine(results):
    total = 0.0
    for r in results:
        o = np.asarray(r["out"], dtype=np.float64)      # [128, NCOL]
        sc = np.asarray(r["out_sc"], dtype=np.float64)  # [2, N_C]
        cols = o.sum(axis=0)
        S = sc[0].sum()
        C = sc[1].sum()
        E = cols[COL_E : COL_E + NB2].sum()
        q_dev = cols[COL_CALQ] / (128 * NCAL)
        q_side = cols[COL_CALQS] / (128 * NCAL)
        # main pass (bf16, all rows) counted-only sums
        x_cnt = S - (N_TOT - C) * THR_B
        e_cnt = E - (N_TOT - C) * q_dev
        # bg rows' side pass (f32) counted-only sums, to subtract
        c_bg = cols[COL_CBG]
        x_cnt_bg = cols[COL_SBG] - (NBG - c_bg) * THR
        e_cnt_bg = cols[COL_EBG] - (NBG - c_bg) * q_side
        c_fg = C - c_bg
        r_fg = (x_cnt - x_cnt_bg) - c_fg * THR
        e_fg = (e_cnt - e_cnt_bg) * math.exp(B_F * THR)
        total += (
            r_fg * (1.0 + C_F)
            + c_fg * (THR + D_F)
            + A_F * e_fg
            + cols[COL_BGT]
            + cols[COL_CORR]
            - cols[COL_GSUM]
        )
    return np.asarray(total / N_I, dtype=np.float32)


def kernel(cls_logits, labels, sel_rand, cat_freq):
    from concourse.bass_utils import run_bass_kernel_spmd

    nc = _get_nc()
    in_maps = _prep_inputs(cls_logits, labels, sel_rand, cat_freq)
    res = run_bass_kernel_spmd(nc, in_maps, core_ids=list(range(N_CORES)))
    return _combine(res.results)
